# revision 1
# baseline (speedup 1.0000x reference)
"""BRF cell (single step) on 8 Trainium2 NeuronCores.

Math (reference, with DT=0.01, THETA=1.0):
    in_sum = x @ W.T
    omega = |omega_p|; p_omega = (-1 + sqrt(1 - (DT*omega)^2)) / DT
    b = p_omega - |b_offset| - 2q
    e = exp(b*DT); c = cos(omega*DT); s = sin(omega*DT)
    u' = e*(u*c - v*s) + in_sum*DT
    v' = e*(u*s + v*c)
    q' = 0.9q + z
    z' = (u' - 1 - q' > 0)

Strategy:
  * The spec fills z and q with zeros. In that case e = exp(DT*(p_omega-|b_off|))
    is a per-neuron constant, q' == 0, and the whole update is a per-neuron
    scaled rotation of (u, v) plus the input projection.
  * Shard the 4096 neurons across the 8 cores (512 each). All big tensors are
    staged TRANSPOSED ([neuron, batch]) so that neurons live on SBUF
    partitions; the per-neuron constants ct = c*e, st = s*e then enter the
    vector ops as per-partition scalars (fused tensor_scalar /
    scalar_tensor_tensor — no broadcast tiles).
  * bf16 I/O: DRAM traffic dominates (memory-bound regime); bf16 halves it.
    Compute is fp32 inside the engines (DVE/ACT are fp32 internally, matmul
    accumulates fp32 in PSUM), only tensor storage is bf16 (~0.2% rounding,
    far inside the tolerance).
  * in_sum.T = (W.T).T @ (DT*x.T) per neuron-block directly in PSUM via the
    TensorEngine; DT is folded into x on the host.
  * Host does only O(N) precompute (omega/b_offset trig) and layout prep
    (transpose/cast/shard); all O(B*N) math runs on the NeuronCores.
  * If z or q is nonzero (never the case for the spec's inputs), fall back to
    an exact fp32 host implementation of the reference.
"""

import numpy as np
import ml_dtypes

DT = 0.01
THETA = 1.0
N_CORES = 8
B = 4096       # batch
N = 4096       # neurons
IN = 256       # input features
NSH = N // N_CORES       # neurons per core
NB = NSH // 128          # 128-partition neuron blocks per core
F = 1024                 # batch-tile (free dim) size
FB = B // F
BF16 = ml_dtypes.bfloat16

_compiled = None


def _build():
    import concourse.bass as bass
    import concourse.tile as tile
    from concourse import bacc, mybir

    nc = bacc.Bacc("TRN2", target_bir_lowering=False, debug=False,
                   num_devices=N_CORES)

    xT = nc.declare_dram_parameter("xT", [IN, B], mybir.dt.bfloat16, isOutput=False)
    WTs = nc.declare_dram_parameter("WTs", [IN, NSH], mybir.dt.bfloat16, isOutput=False)
    uT = nc.declare_dram_parameter("uT", [NSH, B], mybir.dt.bfloat16, isOutput=False)
    vT = nc.declare_dram_parameter("vT", [NSH, B], mybir.dt.bfloat16, isOutput=False)
    cs = nc.declare_dram_parameter("cs", [128, 2 * NB], mybir.dt.float32, isOutput=False)
    unT = nc.declare_dram_parameter("unT", [NSH, B], mybir.dt.bfloat16, isOutput=True)
    vnT = nc.declare_dram_parameter("vnT", [NSH, B], mybir.dt.bfloat16, isOutput=True)
    znT = nc.declare_dram_parameter("znT", [NSH, B], mybir.dt.bfloat16, isOutput=True)

    mult = mybir.AluOpType.mult
    add = mybir.AluOpType.add
    sub = mybir.AluOpType.subtract
    is_gt = mybir.AluOpType.is_gt

    with tile.TileContext(nc) as tc:
        with (
            tc.tile_pool(name="const", bufs=1) as cpool,
            tc.tile_pool(name="io", bufs=3) as iop,
            tc.tile_pool(name="tmp", bufs=3) as tmp,
            tc.tile_pool(name="psum", bufs=3, space=bass.MemorySpace.PSUM) as psp,
        ):
            # Loop-invariant operands.
            xk = []
            for k in range(IN // 128):
                t = cpool.tile([128, B], mybir.dt.bfloat16, tag=f"xk{k}")
                nc.sync.dma_start(t[:], xT[k * 128:(k + 1) * 128, :])
                xk.append(t)
            wk = []
            for k in range(IN // 128):
                t = cpool.tile([128, NSH], mybir.dt.bfloat16, tag=f"wk{k}")
                nc.sync.dma_start(t[:], WTs[k * 128:(k + 1) * 128, :])
                wk.append(t)
            cst = cpool.tile([128, 2 * NB], mybir.dt.float32, tag="cs")
            nc.sync.dma_start(cst[:], cs[:, :])

            for nb in range(NB):
                ct = cst[:, nb:nb + 1]
                st = cst[:, NB + nb:NB + nb + 1]
                nsl = slice(nb * 128, (nb + 1) * 128)
                for fb in range(FB):
                    fsl = slice(fb * F, (fb + 1) * F)
                    # in_sum.T tile: [128 neurons, F batch] fp32 in PSUM.
                    ps = psp.tile([128, F], mybir.dt.float32, tag="ps")
                    for nh in range(F // 512):
                        hsl = slice(nh * 512, (nh + 1) * 512)
                        for k in range(IN // 128):
                            nc.tensor.matmul(
                                ps[:, hsl],
                                wk[k][:, nsl],
                                xk[k][:, fb * F + nh * 512: fb * F + (nh + 1) * 512],
                                start=(k == 0),
                                stop=(k == IN // 128 - 1),
                            )
                    u_t = iop.tile([128, F], mybir.dt.bfloat16, tag="u")
                    nc.sync.dma_start(u_t[:], uT[nsl, fsl])
                    v_t = iop.tile([128, F], mybir.dt.bfloat16, tag="v")
                    nc.sync.dma_start(v_t[:], vT[nsl, fsl])

                    # u' = (u*ct - v*st) + in_sum ; v' = u*st + v*ct
                    t2 = tmp.tile([128, F], mybir.dt.bfloat16, tag="t2")
                    nc.vector.tensor_scalar(t2[:], v_t[:], st, None, mult)
                    un0 = tmp.tile([128, F], mybir.dt.bfloat16, tag="un0")
                    nc.vector.scalar_tensor_tensor(un0[:], u_t[:], ct, t2[:], mult, sub)
                    insb = tmp.tile([128, F], mybir.dt.bfloat16, tag="insb")
                    nc.scalar.copy(insb[:], ps[:])
                    un_t = iop.tile([128, F], mybir.dt.bfloat16, tag="un")
                    nc.vector.tensor_tensor(un_t[:], un0[:], insb[:], add)
                    t3 = tmp.tile([128, F], mybir.dt.bfloat16, tag="t3")
                    nc.vector.tensor_scalar(t3[:], v_t[:], ct, None, mult)
                    vn_t = iop.tile([128, F], mybir.dt.bfloat16, tag="vn")
                    nc.vector.scalar_tensor_tensor(vn_t[:], u_t[:], st, t3[:], mult, add)
                    # z' = ((u' - 1) > 0)   [q' == 0]
                    zn_t = iop.tile([128, F], mybir.dt.bfloat16, tag="zn")
                    nc.vector.tensor_scalar(zn_t[:], un_t[:], float(THETA), 0.0, sub, is_gt)

                    nc.sync.dma_start(unT[nsl, fsl], un_t[:])
                    nc.sync.dma_start(vnT[nsl, fsl], vn_t[:])
                    nc.sync.dma_start(znT[nsl, fsl], zn_t[:])

    nc.compile()
    return nc


def _get_compiled():
    global _compiled
    if _compiled is None:
        _compiled = _build()
    return _compiled


def _prep_in_maps(x, u, v, omega, b_offset):
    om = np.abs(omega.astype(np.float64))
    p_omega = (-1.0 + np.sqrt(1.0 - (DT * om) ** 2)) / DT
    bb = p_omega - np.abs(b_offset.astype(np.float64))
    e = np.exp(DT * bb)
    ct = (np.cos(om * DT) * e).astype(np.float32)
    st = (np.sin(om * DT) * e).astype(np.float32)

    xTd = np.ascontiguousarray(x.T * DT).astype(BF16)      # [IN, B]
    WT = np.ascontiguousarray(np.asarray(W_GLOBAL["W"]).T).astype(BF16)  # [IN, N]
    uT = np.ascontiguousarray(u.T).astype(BF16)            # [N, B]
    vT = np.ascontiguousarray(v.T).astype(BF16)

    in_maps = []
    for i in range(N_CORES):
        sl = slice(i * NSH, (i + 1) * NSH)
        csm = np.empty((128, 2 * NB), np.float32)
        csm[:, 0:NB] = ct[sl].reshape(NB, 128).T
        csm[:, NB:2 * NB] = st[sl].reshape(NB, 128).T
        in_maps.append({
            "xT": xTd,
            "WTs": np.ascontiguousarray(WT[:, sl]),
            "uT": np.ascontiguousarray(uT[sl]),
            "vT": np.ascontiguousarray(vT[sl]),
            "cs": csm,
        })
    return in_maps


W_GLOBAL = {}


def _run_device(x, u, v, W, omega, b_offset, trace=False):
    """Run the fast (z==q==0) path. Returns (z', u', v', exec_time_ns)."""
    from concourse.bass_utils import run_bass_kernel_spmd

    W_GLOBAL["W"] = W
    nc = _get_compiled()
    in_maps = _prep_in_maps(x, u, v, omega, b_offset)
    res = run_bass_kernel_spmd(nc, in_maps, core_ids=list(range(N_CORES)),
                               trace=trace)
    unT = np.concatenate([res.results[i]["unT"] for i in range(N_CORES)], axis=0)
    vnT = np.concatenate([res.results[i]["vnT"] for i in range(N_CORES)], axis=0)
    znT = np.concatenate([res.results[i]["znT"] for i in range(N_CORES)], axis=0)
    u_new = unT.T.astype(np.float32)
    v_new = vnT.T.astype(np.float32)
    z_new = znT.T.astype(np.float32)
    return z_new, u_new, v_new, res.exec_time_ns


def _fallback_host(x, z, u, v, q, W, omega, b_offset):
    """Exact fp32 reference math on the host (only for nonzero z/q inputs)."""
    in_sum = x @ W.T
    om = np.abs(omega)
    p_omega = ((-1.0 + np.sqrt(1.0 - np.square(DT * om))) / DT).astype(np.float32)
    b0 = p_omega - np.abs(b_offset) - q
    bb = b0 - q
    e = np.exp(bb * DT)
    c = np.cos(om * DT)
    s = np.sin(om * DT)
    u_new = e * (u * c - v * s) + in_sum * DT
    v_new = e * (u * s + v * c)
    q_new = 0.9 * q + z
    z_new = (u_new - THETA - q_new > 0).astype(x.dtype)
    return z_new, u_new, v_new, q_new


def kernel(x, z, u, v, q, W, omega, b_offset):
    x = np.asarray(x, np.float32)
    z = np.asarray(z, np.float32)
    u = np.asarray(u, np.float32)
    v = np.asarray(v, np.float32)
    q = np.asarray(q, np.float32)
    W = np.asarray(W, np.float32)
    omega = np.asarray(omega, np.float32)
    b_offset = np.asarray(b_offset, np.float32)

    if z.any() or q.any():
        return _fallback_host(x, z, u, v, q, W, omega, b_offset)

    z_new, u_new, v_new, _ = _run_device(x, u, v, W, omega, b_offset)
    q_new = np.zeros((B, N), np.float32)
    return z_new, u_new, v_new, q_new
